# revision 33
# baseline (speedup 1.0000x reference)
"""Causal self-attention (B=2, S=2048, H=2048, NH=16) on 8 trn2 NeuronCores.

Sharding: core c handles batch b = c//4 and heads [4*(c%4), 4*(c%4)+4).
Each core computes its heads' attention output projected through its rows
of wo (a partial sum of the final output); the host sums the 4 partials
per batch and adds bo.

Device kernel (per core, all matmuls in float32r at full PE rate):
  - q_t/k_t/v_t [HD=128, S] built by streaming xT (host-pretransposed)
    through the PE with per-head weight chunks as the stationary operand.
  - v [S, HD] obtained from v_t via PE transpose.
  - scoresT[k, q] = k_t.T @ q_t per 128x512 tile; softmax runs along the
    PARTITION axis: exp on ACT (no max subtraction needed: |score|<~6),
    denominator = all-ones stationary matmul (partition-sum broadcast to
    all 128 partitions), AV = v.T @ expT -- no transposes of attention
    weights anywhere.
  - normalization (x 1/denom) is fused into the PSUM eviction of outT.
  - fin = sum_h outT_h.T @ wo_h rows, evicted per 128-row tile.
"""

import sys

for _p in ("/opt/trn_rl_repo",):
    if _p not in sys.path:
        sys.path.append(_p)

import numpy as np

import concourse.bacc as bacc
import concourse.bass as bass
import concourse.mybir as mybir
import concourse.tile as tile
from concourse.bass_utils import run_bass_kernel_spmd
from concourse.masks import make_identity
from concourse.tile_rust import add_dep_helper

B, S, H, NH = 2, 2048, 2048, 16
HD = H // NH  # 128
SCALE = float(HD) ** -0.5
HEADS_PER_CORE = 4
N_CORES = 8
NCH = H // 128  # 16 contraction chunks
NSQ = S // 512  # 4 query blocks
NSK = S // 128  # 16 key tiles

F32 = mybir.dt.float32
F32R = mybir.dt.float32r


def _build_nc():
    nc = bacc.Bacc("TRN2", target_bir_lowering=False, debug=False,
                   num_devices=N_CORES)

    xT = nc.declare_dram_parameter("xT", [H, S], F32R, isOutput=False)
    # wq/wk/wv pre-laid-out on host: [h][p][c*128+f] = W[128c+p, 128*head+f]
    wq = nc.declare_dram_parameter("wq", [HEADS_PER_CORE, 128, NCH * 128], F32R, isOutput=False)
    wk = nc.declare_dram_parameter("wk", [HEADS_PER_CORE, 128, NCH * 128], F32R, isOutput=False)
    wv = nc.declare_dram_parameter("wv", [HEADS_PER_CORE, 128, NCH * 128], F32R, isOutput=False)
    wo = nc.declare_dram_parameter("wo", [HEADS_PER_CORE, 128, H], F32R, isOutput=False)
    bq = nc.declare_dram_parameter("bq", [HEADS_PER_CORE, 128, 1], F32, isOutput=False)
    bk = nc.declare_dram_parameter("bk", [HEADS_PER_CORE, 128, 1], F32, isOutput=False)
    bv = nc.declare_dram_parameter("bv", [HEADS_PER_CORE, 128, 1], F32, isOutput=False)
    ones_d = nc.declare_dram_parameter("ones", [128, 128], F32R, isOutput=False)
    fin = nc.declare_dram_parameter("fin", [S, H], F32, isOutput=True)

    Exp = mybir.ActivationFunctionType.Exp
    Ident = mybir.ActivationFunctionType.Identity

    with tile.TileContext(nc) as tc:
        sb = tc.alloc_tile_pool(name="sb", bufs=1)
        ps = tc.alloc_tile_pool(name="ps", bufs=1, space="PSUM")

        # per-head persistent outputs of the attention phase
        outT = [sb.tile([128, S], F32R, tag=f"outT{h}", name=f"outT{h}")
                for h in range(HEADS_PER_CORE)]

        wslots = []  # reuse weight tag slots for wo in the fin phase
        h3_marker = [None]
        for h in range(HEADS_PER_CORE):
            # --- weights + biases for this head -------------------------
            w_tiles = {}
            deferred_w = []
            HW_ = NCH * 128 // 2
            for nm, src in (("wq", wq), ("wk", wk), ("wv", wv)):
                wlo = sb.tile([128, HW_], F32R, tag=nm, bufs=4, name=f"w_{nm}_{h}_lo")
                whi = sb.tile([128, HW_], F32R, tag=nm, bufs=4, name=f"w_{nm}_{h}_hi")
                if h == 0 and nm == "wq":
                    nc.scalar.dma_start(out=wlo[:], in_=src[h][:, :HW_])
                    deferred_w.append((whi, src, HW_))
                elif h == 0:
                    deferred_w.append((wlo, src, 0))
                    deferred_w.append((whi, src, HW_))
                else:
                    nc.sync.dma_start(out=wlo[:], in_=src[h][:, :HW_])
                    nc.sync.dma_start(out=whi[:], in_=src[h][:, HW_:])
                w_tiles[nm] = (wlo, whi)
                if h == 0:
                    wslots.append(nm)
            b_tiles = {}

            # --- projections: q_t/k_t [HD, S] f32r, v_t [HD, S] f32 -----
            q_t = sb.tile([128, S], F32R, tag="q_t")
            k_t = sb.tile([128, S], F32R, tag="k_t")
            v_t = sb.tile([128, S], F32, tag="v_t")
            v_n = sb.tile([128, S], F32R, tag="v_n")
            for q4 in range(4):  # s-quarters of 512
                qp = ps.tile([128, 512], F32, tag="proj", bufs=3, name=f"qp_{h}_{q4}")
                kp = ps.tile([128, 512], F32, tag="proj", bufs=3, name=f"kp_{h}_{q4}")
                vp = ps.tile([128, 512], F32, tag="proj", bufs=3, name=f"vp_{h}_{q4}")
                for c4 in range(NCH // 4):
                    # one DMA stages 4 H-chunks of this s-quarter:
                    # xt[:, 512*i+f] = xT[128*(4*c4+i)+p, 512*q4+f]
                    xt = sb.tile([128, 2048], F32R, tag="xs", bufs=6, name=f"xt_{h}_{q4}_{c4}")
                    nc.sync.dma_start(
                        out=xt[:].rearrange("p (i f) -> p i f", f=512),
                        in_=xT[512 * c4:512 * (c4 + 1), 512 * q4:512 * (q4 + 1)]
                        .rearrange("(i p) f -> p i f", p=128))
                    if h == 0 and q4 == 0 and c4 == 0:
                        for wt_, wsrc_, o_ in deferred_w:
                            nc.scalar.dma_start(out=wt_[:], in_=wsrc_[h][:, o_:o_ + HW_])
                    if h == 0 and q4 == 0 and c4 == 1:
                        ones_t = sb.tile([128, 128], F32R, tag="ones")
                        nc.sync.dma_start(out=ones_t[:], in_=ones_d[:])
                        ident_t = sb.tile([128, 128], F32, tag="ident")
                        make_identity(nc, ident_t[:])
                    if q4 == 0 and c4 == 1:
                        for nm, bsrc in (("bq", bq), ("bk", bk), ("bv", bv)):
                            bt = sb.tile([128, 1], F32, tag=nm, bufs=2, name=f"b_{nm}_{h}")
                            nc.sync.dma_start(out=bt[:], in_=bsrc[h])
                            b_tiles[nm] = bt
                    for i in range(4):
                        c = 4 * c4 + i
                        st, sp = (c == 0), (c == NCH - 1)
                        half, cs = c // 8, bass.ts(c % 8, 128)
                        xs_ = xt[:, bass.ts(i, 512)]
                        nc.tensor.matmul(qp[:], w_tiles["wq"][half][:, cs], xs_, start=st, stop=sp)
                        nc.tensor.matmul(kp[:], w_tiles["wk"][half][:, cs], xs_, start=st, stop=sp)
                        nc.tensor.matmul(vp[:], w_tiles["wv"][half][:, cs], xs_, start=st, stop=sp)
                qs = bass.ts(q4, 512)
                nc.scalar.activation(q_t[:, qs], qp[:], Ident, bias=b_tiles["bq"][:])
                nc.vector.tensor_scalar_add(k_t[:, qs], kp[:], b_tiles["bk"][:])
                nc.scalar.activation(v_t[:, qs], vp[:], Ident, bias=b_tiles["bv"][:])
                # v [S, HD] via PE transpose, interleaved per quarter
                for sk in range(4 * q4, 4 * q4 + 4):
                    tp = ps.tile([128, 128], F32, tag="scores", bufs=3, name=f"vtp_{h}_{sk}")
                    nc.tensor.transpose(tp[:], v_t[:, bass.ts(sk, 128)], ident_t[:])
                    nc.vector.tensor_copy(v_n[:, bass.ts(sk, 128)], tp[:])

            # --- attention, one sq block (512 queries) at a time --------
            for j in range(NSQ):
                nsk = 4 * (j + 1)
                den_p = ps.tile([128, 512], F32, tag="denom", bufs=1, name=f"den_{h}_{j}")
                out_p = ps.tile([128, 512], F32, tag="outT_p", bufs=1, name=f"outp_{h}_{j}")
                exps = [None] * nsk
                qslice = q_t[:, bass.ts(j, 512)]
                # 3-deep software pipeline: scores/exp run ahead of
                # denominator/AV so the PE never waits on ACT.
                LA = 3
                def _off(sk):
                    # valid query columns for this sk tile: [off, 512)
                    o = max(0, 128 * (sk - 4 * j))
                    # a 128-wide f32r matmul runs at 1/4 rate (= full width)
                    return 256 if o == 384 else o

                for i in range(nsk + LA):
                    if i < nsk:
                        sk = i
                        off = _off(sk)
                        w = 512 - off
                        sc = ps.tile([128, 512], F32, tag="scores", bufs=3, name=f"sc_{h}_{j}_{sk}")
                        nc.tensor.matmul(sc[:, off:], k_t[:, bass.ts(sk, 128)],
                                         qslice[:, off:], start=True, stop=True)
                        et = sb.tile([128, 512], F32R, tag="expT", bufs=8, name=f"et_{h}_{j}_{sk}")
                        exp_inst = nc.scalar.activation(et[:, off:], sc[:, off:], Exp, scale=SCALE)
                        if h == 3 and j == 0 and sk == 0:
                            h3_marker[0] = exp_inst.ins
                        if sk >= 4 * j:
                            # diagonal-region tile: zero exp where k > q
                            # (keep where 512j + (off+f) - 128sk - p >= 0)
                            nc.gpsimd.affine_select(
                                out=et[:, off:], in_=et[:, off:],
                                compare_op=mybir.AluOpType.is_ge,
                                fill=0.0, base=512 * j + off - 128 * sk,
                                channel_multiplier=-1, pattern=[[1, w]])
                        exps[sk] = et
                    if i >= LA:
                        sk = i - LA
                        off = _off(sk)
                        st, sp = (sk == 0), (sk == nsk - 1)
                        nc.tensor.matmul(den_p[:, off:], ones_t[:], exps[sk][:, off:],
                                         start=st, stop=sp)
                        nc.tensor.matmul(out_p[:, off:], v_n[:, bass.ts(sk, 128)],
                                         exps[sk][:, off:], start=st, stop=sp)
                # fast ACT evictions free the PSUM accumulators immediately;
                # the DVE recip/normalize chain then runs SBUF-only.
                den_s = sb.tile([128, 512], F32, tag="den_s", bufs=2, name=f"dens_{h}_{j}")
                nc.vector.tensor_copy(den_s[:], den_p[:])
                orw = sb.tile([128, 512], F32, tag="orw", bufs=2, name=f"orw_{h}_{j}")
                nc.vector.tensor_copy(orw[:], out_p[:])
                rec = sb.tile([128, 512], F32, tag="rec", bufs=2, name=f"rec_{h}_{j}")
                nc.vector.reciprocal(rec[:], den_s[:])
                nc.vector.tensor_mul(outT[h][:, bass.ts(j, 512)], orw[:], rec[:])

        # --- final projection: fin[s, :] = sum_h outT_h.T @ wo_h --------
        wo_tiles = []
        HW_ = H // 2
        for h in range(HEADS_PER_CORE):
            pair = []
            for hf in range(2):
                wt = sb.tile([128, HW_], F32R, tag=wslots[(2 * h + hf) % 3],
                             bufs=4, name=f"wo_{h}_{hf}")
                dma = nc.sync.dma_start(out=wt[:], in_=wo[h][:, hf * HW_:(hf + 1) * HW_])
                if h3_marker[0] is not None:
                    add_dep_helper(dma.ins, h3_marker[0], sync=True,
                                   reason="wo load waits for h3 attention start")
                pair.append(wt)
            wo_tiles.append(pair)

        ps.release()
        psf = tc.alloc_tile_pool(name="psf", bufs=1, space="PSUM")

        for s in range(S // 128):
            fo = sb.tile([128, H], F32, tag="fo", bufs=2, name=f"fo_{s}")
            for jb in range(H // 512):
                fp = psf.tile([128, 512], F32, tag="fin", bufs=8, name=f"fp_{s}_{jb}")
                for h in range(HEADS_PER_CORE):
                    nc.tensor.matmul(fp[:], outT[h][:, bass.ts(s, 128)],
                                     wo_tiles[h][jb // 2][:, bass.ts(jb % 2, 512)],
                                     start=(h == 0), stop=(h == HEADS_PER_CORE - 1))
                # alternate eviction engine so neither DVE nor ACT lags PE
                if jb % 2 == 0:
                    nc.vector.tensor_copy(fo[:, bass.ts(jb, 512)], fp[:])
                else:
                    nc.scalar.copy(out=fo[:, bass.ts(jb, 512)], in_=fp[:])
            nc.sync.dma_start(out=fin[bass.ts(s, 128), :H // 2], in_=fo[:, :H // 2])
            nc.sync.dma_start(out=fin[bass.ts(s, 128), H // 2:], in_=fo[:, H // 2:])
        psf.release()
        sb.release()

    nc.finalize()
    return nc


_NC_CACHE = None


def _get_nc():
    global _NC_CACHE
    if _NC_CACHE is None:
        _NC_CACHE = _build_nc()
    return _NC_CACHE


def _prep_inputs(x, mask, wq, bq, wk, bk, wv, bv, wo):
    """Build the 8 per-core input maps (host-side sharding/layout)."""
    x = np.asarray(x, dtype=np.float32)
    mask2d = np.asarray(mask, dtype=np.float32).reshape(S, S)

    xT = [np.ascontiguousarray(x[b].T) for b in range(B)]


    in_maps = []
    for c in range(N_CORES):
        b = c // HEADS_PER_CORE
        g = c % HEADS_PER_CORE
        heads = range(HEADS_PER_CORE * g, HEADS_PER_CORE * (g + 1))
        def _wlayout(w):
            # [h][p][c*128+f] = w[128c+p, 128*head+f]
            return np.stack([np.ascontiguousarray(
                w[:, 128 * h:128 * (h + 1)].reshape(NCH, 128, 128)
                .transpose(1, 0, 2).reshape(128, NCH * 128)) for h in heads])

        wq_c, wk_c, wv_c = _wlayout(wq), _wlayout(wk), _wlayout(wv)
        wo_c = np.ascontiguousarray(
            wo[512 * g:512 * (g + 1), :]).reshape(HEADS_PER_CORE, 128, H)
        bq_c = np.ascontiguousarray(bq[512 * g:512 * (g + 1)]).reshape(HEADS_PER_CORE, 128, 1)
        bk_c = np.ascontiguousarray(bk[512 * g:512 * (g + 1)]).reshape(HEADS_PER_CORE, 128, 1)
        bv_c = np.ascontiguousarray(bv[512 * g:512 * (g + 1)]).reshape(HEADS_PER_CORE, 128, 1)
        in_maps.append({
            "xT": xT[b],
            "wq": wq_c.astype(np.float32), "wk": wk_c.astype(np.float32),
            "wv": wv_c.astype(np.float32), "wo": wo_c.astype(np.float32),
            "bq": bq_c.astype(np.float32), "bk": bk_c.astype(np.float32),
            "bv": bv_c.astype(np.float32),
            "ones": np.ones((128, 128), dtype=np.float32),
        })
    return in_maps


def kernel(x, mask, wq, bq, wk, bk, wv, bv, wo, bo, _trace=False):
    nc = _get_nc()
    x, mask, wq, bq, wk, bk, wv, bv, wo, bo = (
        np.asarray(t, dtype=np.float32)
        for t in (x, mask, wq, bq, wk, bk, wv, bv, wo, bo))
    in_maps = _prep_inputs(x, mask, wq, bq, wk, bk, wv, bv, wo)
    res = run_bass_kernel_spmd(nc, in_maps, list(range(N_CORES)), trace=_trace)

    out = np.empty((B, S, H), dtype=np.float32)
    for b in range(B):
        acc = np.zeros((S, H), dtype=np.float32)
        for g in range(HEADS_PER_CORE):
            acc += res.results[b * HEADS_PER_CORE + g]["fin"]
        out[b] = acc + bo[None, :]
    kernel.last_exec_time_ns = res.exec_time_ns
    return out


kernel.last_exec_time_ns = None


# revision 34
# speedup vs baseline: 1.0017x; 1.0017x over previous
"""Causal self-attention (B=2, S=2048, H=2048, NH=16) on 8 trn2 NeuronCores.

Sharding: core c handles batch b = c//4 and heads [4*(c%4), 4*(c%4)+4).
Each core computes its heads' attention output projected through its rows
of wo (a partial sum of the final output); the host sums the 4 partials
per batch and adds bo.

Device kernel (per core, all matmuls in float32r at full PE rate):
  - q_t/k_t/v_t [HD=128, S] built by streaming xT (host-pretransposed)
    through the PE with per-head weight chunks as the stationary operand.
  - v [S, HD] obtained from v_t via PE transpose.
  - scoresT[k, q] = k_t.T @ q_t per 128x512 tile; softmax runs along the
    PARTITION axis: exp on ACT (no max subtraction needed: |score|<~6),
    denominator = all-ones stationary matmul (partition-sum broadcast to
    all 128 partitions), AV = v.T @ expT -- no transposes of attention
    weights anywhere.
  - normalization (x 1/denom) is fused into the PSUM eviction of outT.
  - fin = sum_h outT_h.T @ wo_h rows, evicted per 128-row tile.
"""

import sys

for _p in ("/opt/trn_rl_repo",):
    if _p not in sys.path:
        sys.path.append(_p)

import numpy as np

import concourse.bacc as bacc
import concourse.bass as bass
import concourse.mybir as mybir
import concourse.tile as tile
from concourse.bass_utils import run_bass_kernel_spmd
from concourse.masks import make_identity
from concourse.tile_rust import add_dep_helper

B, S, H, NH = 2, 2048, 2048, 16
HD = H // NH  # 128
SCALE = float(HD) ** -0.5
HEADS_PER_CORE = 4
N_CORES = 8
NCH = H // 128  # 16 contraction chunks
NSQ = S // 512  # 4 query blocks
NSK = S // 128  # 16 key tiles

F32 = mybir.dt.float32
F32R = mybir.dt.float32r


def _build_nc():
    nc = bacc.Bacc("TRN2", target_bir_lowering=False, debug=False,
                   num_devices=N_CORES)

    xT = nc.declare_dram_parameter("xT", [H, S], F32R, isOutput=False)
    # wq/wk/wv pre-laid-out on host: [h][p][c*128+f] = W[128c+p, 128*head+f]
    wq = nc.declare_dram_parameter("wq", [HEADS_PER_CORE, 128, NCH * 128], F32R, isOutput=False)
    wk = nc.declare_dram_parameter("wk", [HEADS_PER_CORE, 128, NCH * 128], F32R, isOutput=False)
    wv = nc.declare_dram_parameter("wv", [HEADS_PER_CORE, 128, NCH * 128], F32R, isOutput=False)
    wo = nc.declare_dram_parameter("wo", [HEADS_PER_CORE, 128, H], F32R, isOutput=False)
    bq = nc.declare_dram_parameter("bq", [HEADS_PER_CORE, 128, 1], F32, isOutput=False)
    bk = nc.declare_dram_parameter("bk", [HEADS_PER_CORE, 128, 1], F32, isOutput=False)
    bv = nc.declare_dram_parameter("bv", [HEADS_PER_CORE, 128, 1], F32, isOutput=False)
    ones_d = nc.declare_dram_parameter("ones", [128, 128], F32R, isOutput=False)
    fin = nc.declare_dram_parameter("fin", [S, H], F32, isOutput=True)

    Exp = mybir.ActivationFunctionType.Exp
    Ident = mybir.ActivationFunctionType.Identity

    with tile.TileContext(nc) as tc:
        sb = tc.alloc_tile_pool(name="sb", bufs=1)
        ps = tc.alloc_tile_pool(name="ps", bufs=1, space="PSUM")

        # per-head persistent outputs of the attention phase
        outT = [sb.tile([128, S], F32R, tag=f"outT{h}", name=f"outT{h}")
                for h in range(HEADS_PER_CORE)]

        wslots = []  # reuse weight tag slots for wo in the fin phase
        h3_marker = [None]
        for h in range(HEADS_PER_CORE):
            # --- weights + biases for this head -------------------------
            w_tiles = {}
            deferred_w = []
            HW_ = NCH * 128 // 2
            for nm, src in (("wq", wq), ("wk", wk), ("wv", wv)):
                wlo = sb.tile([128, HW_], F32R, tag=nm, bufs=4, name=f"w_{nm}_{h}_lo")
                whi = sb.tile([128, HW_], F32R, tag=nm, bufs=4, name=f"w_{nm}_{h}_hi")
                if h == 0 and nm == "wq":
                    nc.sync.dma_start(out=wlo[:], in_=src[h][:, :HW_])
                    deferred_w.append((whi, src, HW_))
                elif h == 0:
                    deferred_w.append((wlo, src, 0))
                    deferred_w.append((whi, src, HW_))
                else:
                    nc.sync.dma_start(out=wlo[:], in_=src[h][:, :HW_])
                    nc.sync.dma_start(out=whi[:], in_=src[h][:, HW_:])
                w_tiles[nm] = (wlo, whi)
                if h == 0:
                    wslots.append(nm)
            b_tiles = {}

            # --- projections: q_t/k_t [HD, S] f32r, v_t [HD, S] f32 -----
            q_t = sb.tile([128, S], F32R, tag="q_t")
            k_t = sb.tile([128, S], F32R, tag="k_t")
            v_t = sb.tile([128, S], F32, tag="v_t")
            v_n = sb.tile([128, S], F32R, tag="v_n")
            for q4 in range(4):  # s-quarters of 512
                qp = ps.tile([128, 512], F32, tag="proj", bufs=3, name=f"qp_{h}_{q4}")
                kp = ps.tile([128, 512], F32, tag="proj", bufs=3, name=f"kp_{h}_{q4}")
                vp = ps.tile([128, 512], F32, tag="proj", bufs=3, name=f"vp_{h}_{q4}")
                for c4 in range(NCH // 4):
                    # one DMA stages 4 H-chunks of this s-quarter:
                    # xt[:, 512*i+f] = xT[128*(4*c4+i)+p, 512*q4+f]
                    xt = sb.tile([128, 2048], F32R, tag="xs", bufs=6, name=f"xt_{h}_{q4}_{c4}")
                    nc.sync.dma_start(
                        out=xt[:].rearrange("p (i f) -> p i f", f=512),
                        in_=xT[512 * c4:512 * (c4 + 1), 512 * q4:512 * (q4 + 1)]
                        .rearrange("(i p) f -> p i f", p=128))
                    if h == 0 and q4 == 0 and c4 == 0:
                        for wt_, wsrc_, o_ in deferred_w:
                            nc.sync.dma_start(out=wt_[:], in_=wsrc_[h][:, o_:o_ + HW_])
                    if h == 0 and q4 == 0 and c4 == 1:
                        ones_t = sb.tile([128, 128], F32R, tag="ones")
                        nc.sync.dma_start(out=ones_t[:], in_=ones_d[:])
                        ident_t = sb.tile([128, 128], F32, tag="ident")
                        make_identity(nc, ident_t[:])
                    if q4 == 0 and c4 == 1:
                        for nm, bsrc in (("bq", bq), ("bk", bk), ("bv", bv)):
                            bt = sb.tile([128, 1], F32, tag=nm, bufs=2, name=f"b_{nm}_{h}")
                            nc.sync.dma_start(out=bt[:], in_=bsrc[h])
                            b_tiles[nm] = bt
                    for i in range(4):
                        c = 4 * c4 + i
                        st, sp = (c == 0), (c == NCH - 1)
                        half, cs = c // 8, bass.ts(c % 8, 128)
                        xs_ = xt[:, bass.ts(i, 512)]
                        nc.tensor.matmul(qp[:], w_tiles["wq"][half][:, cs], xs_, start=st, stop=sp)
                        nc.tensor.matmul(kp[:], w_tiles["wk"][half][:, cs], xs_, start=st, stop=sp)
                        nc.tensor.matmul(vp[:], w_tiles["wv"][half][:, cs], xs_, start=st, stop=sp)
                qs = bass.ts(q4, 512)
                nc.scalar.activation(q_t[:, qs], qp[:], Ident, bias=b_tiles["bq"][:])
                nc.vector.tensor_scalar_add(k_t[:, qs], kp[:], b_tiles["bk"][:])
                nc.scalar.activation(v_t[:, qs], vp[:], Ident, bias=b_tiles["bv"][:])
                # v [S, HD] via PE transpose, interleaved per quarter
                for sk in range(4 * q4, 4 * q4 + 4):
                    tp = ps.tile([128, 128], F32, tag="scores", bufs=3, name=f"vtp_{h}_{sk}")
                    nc.tensor.transpose(tp[:], v_t[:, bass.ts(sk, 128)], ident_t[:])
                    nc.vector.tensor_copy(v_n[:, bass.ts(sk, 128)], tp[:])

            # --- attention, one sq block (512 queries) at a time --------
            for j in range(NSQ):
                nsk = 4 * (j + 1)
                den_p = ps.tile([128, 512], F32, tag="denom", bufs=1, name=f"den_{h}_{j}")
                out_p = ps.tile([128, 512], F32, tag="outT_p", bufs=1, name=f"outp_{h}_{j}")
                exps = [None] * nsk
                qslice = q_t[:, bass.ts(j, 512)]
                # 3-deep software pipeline: scores/exp run ahead of
                # denominator/AV so the PE never waits on ACT.
                LA = 3
                def _off(sk):
                    # valid query columns for this sk tile: [off, 512)
                    o = max(0, 128 * (sk - 4 * j))
                    # a 128-wide f32r matmul runs at 1/4 rate (= full width)
                    return 256 if o == 384 else o

                for i in range(nsk + LA):
                    if i < nsk:
                        sk = i
                        off = _off(sk)
                        w = 512 - off
                        sc = ps.tile([128, 512], F32, tag="scores", bufs=3, name=f"sc_{h}_{j}_{sk}")
                        nc.tensor.matmul(sc[:, off:], k_t[:, bass.ts(sk, 128)],
                                         qslice[:, off:], start=True, stop=True)
                        et = sb.tile([128, 512], F32R, tag="expT", bufs=8, name=f"et_{h}_{j}_{sk}")
                        exp_inst = nc.scalar.activation(et[:, off:], sc[:, off:], Exp, scale=SCALE)
                        if h == 3 and j == 0 and sk == 0:
                            h3_marker[0] = exp_inst.ins
                        if sk >= 4 * j:
                            # diagonal-region tile: zero exp where k > q
                            # (keep where 512j + (off+f) - 128sk - p >= 0)
                            nc.gpsimd.affine_select(
                                out=et[:, off:], in_=et[:, off:],
                                compare_op=mybir.AluOpType.is_ge,
                                fill=0.0, base=512 * j + off - 128 * sk,
                                channel_multiplier=-1, pattern=[[1, w]])
                        exps[sk] = et
                    if i >= LA:
                        sk = i - LA
                        off = _off(sk)
                        st, sp = (sk == 0), (sk == nsk - 1)
                        nc.tensor.matmul(den_p[:, off:], ones_t[:], exps[sk][:, off:],
                                         start=st, stop=sp)
                        nc.tensor.matmul(out_p[:, off:], v_n[:, bass.ts(sk, 128)],
                                         exps[sk][:, off:], start=st, stop=sp)
                # fast ACT evictions free the PSUM accumulators immediately;
                # the DVE recip/normalize chain then runs SBUF-only.
                den_s = sb.tile([128, 512], F32, tag="den_s", bufs=2, name=f"dens_{h}_{j}")
                nc.vector.tensor_copy(den_s[:], den_p[:])
                orw = sb.tile([128, 512], F32, tag="orw", bufs=2, name=f"orw_{h}_{j}")
                nc.vector.tensor_copy(orw[:], out_p[:])
                rec = sb.tile([128, 512], F32, tag="rec", bufs=2, name=f"rec_{h}_{j}")
                nc.vector.reciprocal(rec[:], den_s[:])
                nc.vector.tensor_mul(outT[h][:, bass.ts(j, 512)], orw[:], rec[:])

        # --- final projection: fin[s, :] = sum_h outT_h.T @ wo_h --------
        wo_tiles = []
        HW_ = H // 2
        for h in range(HEADS_PER_CORE):
            pair = []
            for hf in range(2):
                wt = sb.tile([128, HW_], F32R, tag=wslots[(2 * h + hf) % 3],
                             bufs=4, name=f"wo_{h}_{hf}")
                dma = nc.sync.dma_start(out=wt[:], in_=wo[h][:, hf * HW_:(hf + 1) * HW_])
                if h3_marker[0] is not None:
                    add_dep_helper(dma.ins, h3_marker[0], sync=True,
                                   reason="wo load waits for h3 attention start")
                pair.append(wt)
            wo_tiles.append(pair)

        ps.release()
        psf = tc.alloc_tile_pool(name="psf", bufs=1, space="PSUM")

        for s in range(S // 128):
            fo = sb.tile([128, H], F32, tag="fo", bufs=2, name=f"fo_{s}")
            for jb in range(H // 512):
                fp = psf.tile([128, 512], F32, tag="fin", bufs=8, name=f"fp_{s}_{jb}")
                for h in range(HEADS_PER_CORE):
                    nc.tensor.matmul(fp[:], outT[h][:, bass.ts(s, 128)],
                                     wo_tiles[h][jb // 2][:, bass.ts(jb % 2, 512)],
                                     start=(h == 0), stop=(h == HEADS_PER_CORE - 1))
                # alternate eviction engine so neither DVE nor ACT lags PE
                if jb % 2 == 0:
                    nc.vector.tensor_copy(fo[:, bass.ts(jb, 512)], fp[:])
                else:
                    nc.scalar.copy(out=fo[:, bass.ts(jb, 512)], in_=fp[:])
            nc.sync.dma_start(out=fin[bass.ts(s, 128), :], in_=fo[:])
        psf.release()
        sb.release()

    nc.finalize()
    return nc


_NC_CACHE = None


def _get_nc():
    global _NC_CACHE
    if _NC_CACHE is None:
        _NC_CACHE = _build_nc()
    return _NC_CACHE


def _prep_inputs(x, mask, wq, bq, wk, bk, wv, bv, wo):
    """Build the 8 per-core input maps (host-side sharding/layout)."""
    x = np.asarray(x, dtype=np.float32)
    mask2d = np.asarray(mask, dtype=np.float32).reshape(S, S)

    xT = [np.ascontiguousarray(x[b].T) for b in range(B)]


    in_maps = []
    for c in range(N_CORES):
        b = c // HEADS_PER_CORE
        g = c % HEADS_PER_CORE
        heads = range(HEADS_PER_CORE * g, HEADS_PER_CORE * (g + 1))
        def _wlayout(w):
            # [h][p][c*128+f] = w[128c+p, 128*head+f]
            return np.stack([np.ascontiguousarray(
                w[:, 128 * h:128 * (h + 1)].reshape(NCH, 128, 128)
                .transpose(1, 0, 2).reshape(128, NCH * 128)) for h in heads])

        wq_c, wk_c, wv_c = _wlayout(wq), _wlayout(wk), _wlayout(wv)
        wo_c = np.ascontiguousarray(
            wo[512 * g:512 * (g + 1), :]).reshape(HEADS_PER_CORE, 128, H)
        bq_c = np.ascontiguousarray(bq[512 * g:512 * (g + 1)]).reshape(HEADS_PER_CORE, 128, 1)
        bk_c = np.ascontiguousarray(bk[512 * g:512 * (g + 1)]).reshape(HEADS_PER_CORE, 128, 1)
        bv_c = np.ascontiguousarray(bv[512 * g:512 * (g + 1)]).reshape(HEADS_PER_CORE, 128, 1)
        in_maps.append({
            "xT": xT[b],
            "wq": wq_c.astype(np.float32), "wk": wk_c.astype(np.float32),
            "wv": wv_c.astype(np.float32), "wo": wo_c.astype(np.float32),
            "bq": bq_c.astype(np.float32), "bk": bk_c.astype(np.float32),
            "bv": bv_c.astype(np.float32),
            "ones": np.ones((128, 128), dtype=np.float32),
        })
    return in_maps


def kernel(x, mask, wq, bq, wk, bk, wv, bv, wo, bo, _trace=False):
    nc = _get_nc()
    x, mask, wq, bq, wk, bk, wv, bv, wo, bo = (
        np.asarray(t, dtype=np.float32)
        for t in (x, mask, wq, bq, wk, bk, wv, bv, wo, bo))
    in_maps = _prep_inputs(x, mask, wq, bq, wk, bk, wv, bv, wo)
    res = run_bass_kernel_spmd(nc, in_maps, list(range(N_CORES)), trace=_trace)

    out = np.empty((B, S, H), dtype=np.float32)
    for b in range(B):
        acc = np.zeros((S, H), dtype=np.float32)
        for g in range(HEADS_PER_CORE):
            acc += res.results[b * HEADS_PER_CORE + g]["fin"]
        out[b] = acc + bo[None, :]
    kernel.last_exec_time_ns = res.exec_time_ns
    return out


kernel.last_exec_time_ns = None


# revision 35
# speedup vs baseline: 1.0305x; 1.0287x over previous
"""Causal self-attention (B=2, S=2048, H=2048, NH=16) on 8 trn2 NeuronCores.

Sharding: core c handles batch b = c//4 and heads [4*(c%4), 4*(c%4)+4).
Each core computes its heads' attention output projected through its rows
of wo (a partial sum of the final output); the host sums the 4 partials
per batch and adds bo.

Device kernel (per core, all matmuls in float32r at full PE rate):
  - q_t/k_t/v_t [HD=128, S] built by streaming xT (host-pretransposed)
    through the PE with per-head weight chunks as the stationary operand.
  - v [S, HD] obtained from v_t via PE transpose.
  - scoresT[k, q] = k_t.T @ q_t per 128x512 tile; softmax runs along the
    PARTITION axis: exp on ACT (no max subtraction needed: |score|<~6),
    denominator = all-ones stationary matmul (partition-sum broadcast to
    all 128 partitions), AV = v.T @ expT -- no transposes of attention
    weights anywhere.
  - normalization (x 1/denom) is fused into the PSUM eviction of outT.
  - fin = sum_h outT_h.T @ wo_h rows, evicted per 128-row tile.
"""

import sys

for _p in ("/opt/trn_rl_repo",):
    if _p not in sys.path:
        sys.path.append(_p)

import numpy as np

import concourse.bacc as bacc
import concourse.bass as bass
import concourse.mybir as mybir
import concourse.tile as tile
from concourse.bass_utils import run_bass_kernel_spmd
from concourse.masks import make_identity
from concourse.tile_rust import add_dep_helper

B, S, H, NH = 2, 2048, 2048, 16
HD = H // NH  # 128
SCALE = float(HD) ** -0.5
HEADS_PER_CORE = 4
N_CORES = 8
NCH = H // 128  # 16 contraction chunks
NSQ = S // 512  # 4 query blocks
NSK = S // 128  # 16 key tiles

F32 = mybir.dt.float32
F32R = mybir.dt.float32r


def _build_nc():
    nc = bacc.Bacc("TRN2", target_bir_lowering=False, debug=False,
                   num_devices=N_CORES)

    xT = nc.declare_dram_parameter("xT", [H, S], F32R, isOutput=False)
    # wq/wk/wv pre-laid-out on host: [h][p][c*128+f] = W[128c+p, 128*head+f]
    wq = nc.declare_dram_parameter("wq", [HEADS_PER_CORE, 128, NCH * 128], F32R, isOutput=False)
    wk = nc.declare_dram_parameter("wk", [HEADS_PER_CORE, 128, NCH * 128], F32R, isOutput=False)
    wv = nc.declare_dram_parameter("wv", [HEADS_PER_CORE, 128, NCH * 128], F32R, isOutput=False)
    wo = nc.declare_dram_parameter("wo", [HEADS_PER_CORE, 128, H], F32R, isOutput=False)
    bq = nc.declare_dram_parameter("bq", [HEADS_PER_CORE, 128, 1], F32, isOutput=False)
    bk = nc.declare_dram_parameter("bk", [HEADS_PER_CORE, 128, 1], F32, isOutput=False)
    bv = nc.declare_dram_parameter("bv", [HEADS_PER_CORE, 128, 1], F32, isOutput=False)
    ones_d = nc.declare_dram_parameter("ones", [128, 128], F32R, isOutput=False)
    fin = nc.declare_dram_parameter("fin", [S, H], F32, isOutput=True)

    Exp = mybir.ActivationFunctionType.Exp
    Ident = mybir.ActivationFunctionType.Identity

    with tile.TileContext(nc) as tc:
        sb = tc.alloc_tile_pool(name="sb", bufs=1)
        ps = tc.alloc_tile_pool(name="ps", bufs=1, space="PSUM")

        # per-head persistent outputs of the attention phase
        outT = [sb.tile([128, S], F32R, tag=f"outT{h}", name=f"outT{h}")
                for h in range(HEADS_PER_CORE)]

        wslots = []  # reuse weight tag slots for wo in the fin phase
        h3_marker = [None]
        for h in range(HEADS_PER_CORE):
            # --- weights + biases for this head -------------------------
            w_tiles = {}
            deferred_w = []
            HW_ = NCH * 128 // 2
            for nm, src in (("wq", wq), ("wk", wk), ("wv", wv)):
                wlo = sb.tile([128, HW_], F32R, tag=nm, bufs=4, name=f"w_{nm}_{h}_lo")
                whi = sb.tile([128, HW_], F32R, tag=nm, bufs=4, name=f"w_{nm}_{h}_hi")
                if h == 0 and nm == "wq":
                    nc.sync.dma_start(out=wlo[:], in_=src[h][:, :HW_])
                    deferred_w.append((whi, src, HW_))
                elif h == 0:
                    deferred_w.append((wlo, src, 0))
                    deferred_w.append((whi, src, HW_))
                else:
                    nc.sync.dma_start(out=wlo[:], in_=src[h][:, :HW_])
                    nc.sync.dma_start(out=whi[:], in_=src[h][:, HW_:])
                w_tiles[nm] = (wlo, whi)
                if h == 0:
                    wslots.append(nm)
            b_tiles = {}

            # --- projections: q_t/k_t [HD, S] f32r, v_t [HD, S] f32 -----
            q_t = sb.tile([128, S], F32R, tag="q_t")
            k_t = sb.tile([128, S], F32R, tag="k_t")
            v_t = sb.tile([128, S], F32, tag="v_t")
            v_n = sb.tile([128, S], F32R, tag="v_n")
            # --- attention for sq block j: needs proj quarters <= j ------
            def emit_attn(j):
                nsk = 4 * (j + 1)
                den_p = ps.tile([128, 512], F32, tag="denom", bufs=1, name=f"den_{h}_{j}")
                out_p = ps.tile([128, 512], F32, tag="outT_p", bufs=1, name=f"outp_{h}_{j}")
                exps = [None] * nsk
                qslice = q_t[:, bass.ts(j, 512)]
                # 3-deep software pipeline: scores/exp run ahead of
                # denominator/AV so the PE never waits on ACT.
                LA = 3
                def _off(sk):
                    # valid query columns for this sk tile: [off, 512)
                    o = max(0, 128 * (sk - 4 * j))
                    # a 128-wide f32r matmul runs at 1/4 rate (= full width)
                    return 256 if o == 384 else o

                for i in range(nsk + LA):
                    if i < nsk:
                        sk = i
                        off = _off(sk)
                        w = 512 - off
                        sc = ps.tile([128, 512], F32, tag="scores", bufs=3, name=f"sc_{h}_{j}_{sk}")
                        nc.tensor.matmul(sc[:, off:], k_t[:, bass.ts(sk, 128)],
                                         qslice[:, off:], start=True, stop=True)
                        et = sb.tile([128, 512], F32R, tag="expT", bufs=8, name=f"et_{h}_{j}_{sk}")
                        exp_inst = nc.scalar.activation(et[:, off:], sc[:, off:], Exp, scale=SCALE)
                        if h == 3 and j == 0 and sk == 0:
                            h3_marker[0] = exp_inst.ins
                        if sk >= 4 * j:
                            # diagonal-region tile: zero exp where k > q
                            # (keep where 512j + (off+f) - 128sk - p >= 0)
                            nc.gpsimd.affine_select(
                                out=et[:, off:], in_=et[:, off:],
                                compare_op=mybir.AluOpType.is_ge,
                                fill=0.0, base=512 * j + off - 128 * sk,
                                channel_multiplier=-1, pattern=[[1, w]])
                        exps[sk] = et
                    if i >= LA:
                        sk = i - LA
                        off = _off(sk)
                        st, sp = (sk == 0), (sk == nsk - 1)
                        nc.tensor.matmul(den_p[:, off:], ones_t[:], exps[sk][:, off:],
                                         start=st, stop=sp)
                        nc.tensor.matmul(out_p[:, off:], v_n[:, bass.ts(sk, 128)],
                                         exps[sk][:, off:], start=st, stop=sp)
                # fast ACT evictions free the PSUM accumulators immediately;
                # the DVE recip/normalize chain then runs SBUF-only.
                den_s = sb.tile([128, 512], F32, tag="den_s", bufs=2, name=f"dens_{h}_{j}")
                nc.vector.tensor_copy(den_s[:], den_p[:])
                orw = sb.tile([128, 512], F32, tag="orw", bufs=2, name=f"orw_{h}_{j}")
                nc.vector.tensor_copy(orw[:], out_p[:])
                rec = sb.tile([128, 512], F32, tag="rec", bufs=2, name=f"rec_{h}_{j}")
                nc.vector.reciprocal(rec[:], den_s[:])
                nc.vector.tensor_mul(outT[h][:, bass.ts(j, 512)], orw[:], rec[:])


            for q4 in range(4):  # s-quarters of 512
                qp = ps.tile([128, 512], F32, tag="proj", bufs=3, name=f"qp_{h}_{q4}")
                kp = ps.tile([128, 512], F32, tag="proj", bufs=3, name=f"kp_{h}_{q4}")
                vp = ps.tile([128, 512], F32, tag="proj", bufs=3, name=f"vp_{h}_{q4}")
                for c4 in range(NCH // 4):
                    # one DMA stages 4 H-chunks of this s-quarter:
                    # xt[:, 512*i+f] = xT[128*(4*c4+i)+p, 512*q4+f]
                    xt = sb.tile([128, 2048], F32R, tag="xs", bufs=6, name=f"xt_{h}_{q4}_{c4}")
                    nc.sync.dma_start(
                        out=xt[:].rearrange("p (i f) -> p i f", f=512),
                        in_=xT[512 * c4:512 * (c4 + 1), 512 * q4:512 * (q4 + 1)]
                        .rearrange("(i p) f -> p i f", p=128))
                    if h == 0 and q4 == 0 and c4 == 0:
                        for wt_, wsrc_, o_ in deferred_w:
                            nc.sync.dma_start(out=wt_[:], in_=wsrc_[h][:, o_:o_ + HW_])
                    if h == 0 and q4 == 0 and c4 == 1:
                        ones_t = sb.tile([128, 128], F32R, tag="ones")
                        nc.sync.dma_start(out=ones_t[:], in_=ones_d[:])
                        ident_t = sb.tile([128, 128], F32, tag="ident")
                        make_identity(nc, ident_t[:])
                    if q4 == 0 and c4 == 1:
                        for nm, bsrc in (("bq", bq), ("bk", bk), ("bv", bv)):
                            bt = sb.tile([128, 1], F32, tag=nm, bufs=2, name=f"b_{nm}_{h}")
                            nc.sync.dma_start(out=bt[:], in_=bsrc[h])
                            b_tiles[nm] = bt
                    for i in range(4):
                        c = 4 * c4 + i
                        st, sp = (c == 0), (c == NCH - 1)
                        half, cs = c // 8, bass.ts(c % 8, 128)
                        xs_ = xt[:, bass.ts(i, 512)]
                        nc.tensor.matmul(qp[:], w_tiles["wq"][half][:, cs], xs_, start=st, stop=sp)
                        nc.tensor.matmul(kp[:], w_tiles["wk"][half][:, cs], xs_, start=st, stop=sp)
                        nc.tensor.matmul(vp[:], w_tiles["wv"][half][:, cs], xs_, start=st, stop=sp)
                qs = bass.ts(q4, 512)
                nc.scalar.activation(q_t[:, qs], qp[:], Ident, bias=b_tiles["bq"][:])
                nc.vector.tensor_scalar_add(k_t[:, qs], kp[:], b_tiles["bk"][:])
                nc.scalar.activation(v_t[:, qs], vp[:], Ident, bias=b_tiles["bv"][:])
                # v [S, HD] via PE transpose, interleaved per quarter
                for sk in range(4 * q4, 4 * q4 + 4):
                    tp = ps.tile([128, 128], F32, tag="scores", bufs=3, name=f"vtp_{h}_{sk}")
                    nc.tensor.transpose(tp[:], v_t[:, bass.ts(sk, 128)], ident_t[:])
                    nc.vector.tensor_copy(v_n[:, bass.ts(sk, 128)], tp[:])
                emit_attn(q4)

        # --- final projection: fin[s, :] = sum_h outT_h.T @ wo_h --------
        wo_tiles = []
        HW_ = H // 2
        for h in range(HEADS_PER_CORE):
            pair = []
            for hf in range(2):
                wt = sb.tile([128, HW_], F32R, tag=wslots[(2 * h + hf) % 3],
                             bufs=4, name=f"wo_{h}_{hf}")
                dma = nc.sync.dma_start(out=wt[:], in_=wo[h][:, hf * HW_:(hf + 1) * HW_])
                if h3_marker[0] is not None:
                    add_dep_helper(dma.ins, h3_marker[0], sync=True,
                                   reason="wo load waits for h3 attention start")
                pair.append(wt)
            wo_tiles.append(pair)

        ps.release()
        psf = tc.alloc_tile_pool(name="psf", bufs=1, space="PSUM")

        for s in range(S // 128):
            fo = sb.tile([128, H], F32, tag="fo", bufs=2, name=f"fo_{s}")
            for jb in range(H // 512):
                fp = psf.tile([128, 512], F32, tag="fin", bufs=8, name=f"fp_{s}_{jb}")
                for h in range(HEADS_PER_CORE):
                    nc.tensor.matmul(fp[:], outT[h][:, bass.ts(s, 128)],
                                     wo_tiles[h][jb // 2][:, bass.ts(jb % 2, 512)],
                                     start=(h == 0), stop=(h == HEADS_PER_CORE - 1))
                # alternate eviction engine so neither DVE nor ACT lags PE
                if jb % 2 == 0:
                    nc.vector.tensor_copy(fo[:, bass.ts(jb, 512)], fp[:])
                else:
                    nc.scalar.copy(out=fo[:, bass.ts(jb, 512)], in_=fp[:])
            nc.sync.dma_start(out=fin[bass.ts(s, 128), :], in_=fo[:])
        psf.release()
        sb.release()

    nc.finalize()
    return nc


_NC_CACHE = None


def _get_nc():
    global _NC_CACHE
    if _NC_CACHE is None:
        _NC_CACHE = _build_nc()
    return _NC_CACHE


def _prep_inputs(x, mask, wq, bq, wk, bk, wv, bv, wo):
    """Build the 8 per-core input maps (host-side sharding/layout)."""
    x = np.asarray(x, dtype=np.float32)
    mask2d = np.asarray(mask, dtype=np.float32).reshape(S, S)

    xT = [np.ascontiguousarray(x[b].T) for b in range(B)]


    in_maps = []
    for c in range(N_CORES):
        b = c // HEADS_PER_CORE
        g = c % HEADS_PER_CORE
        heads = range(HEADS_PER_CORE * g, HEADS_PER_CORE * (g + 1))
        def _wlayout(w):
            # [h][p][c*128+f] = w[128c+p, 128*head+f]
            return np.stack([np.ascontiguousarray(
                w[:, 128 * h:128 * (h + 1)].reshape(NCH, 128, 128)
                .transpose(1, 0, 2).reshape(128, NCH * 128)) for h in heads])

        wq_c, wk_c, wv_c = _wlayout(wq), _wlayout(wk), _wlayout(wv)
        wo_c = np.ascontiguousarray(
            wo[512 * g:512 * (g + 1), :]).reshape(HEADS_PER_CORE, 128, H)
        bq_c = np.ascontiguousarray(bq[512 * g:512 * (g + 1)]).reshape(HEADS_PER_CORE, 128, 1)
        bk_c = np.ascontiguousarray(bk[512 * g:512 * (g + 1)]).reshape(HEADS_PER_CORE, 128, 1)
        bv_c = np.ascontiguousarray(bv[512 * g:512 * (g + 1)]).reshape(HEADS_PER_CORE, 128, 1)
        in_maps.append({
            "xT": xT[b],
            "wq": wq_c.astype(np.float32), "wk": wk_c.astype(np.float32),
            "wv": wv_c.astype(np.float32), "wo": wo_c.astype(np.float32),
            "bq": bq_c.astype(np.float32), "bk": bk_c.astype(np.float32),
            "bv": bv_c.astype(np.float32),
            "ones": np.ones((128, 128), dtype=np.float32),
        })
    return in_maps


def kernel(x, mask, wq, bq, wk, bk, wv, bv, wo, bo, _trace=False):
    nc = _get_nc()
    x, mask, wq, bq, wk, bk, wv, bv, wo, bo = (
        np.asarray(t, dtype=np.float32)
        for t in (x, mask, wq, bq, wk, bk, wv, bv, wo, bo))
    in_maps = _prep_inputs(x, mask, wq, bq, wk, bk, wv, bv, wo)
    res = run_bass_kernel_spmd(nc, in_maps, list(range(N_CORES)), trace=_trace)

    out = np.empty((B, S, H), dtype=np.float32)
    for b in range(B):
        acc = np.zeros((S, H), dtype=np.float32)
        for g in range(HEADS_PER_CORE):
            acc += res.results[b * HEADS_PER_CORE + g]["fin"]
        out[b] = acc + bo[None, :]
    kernel.last_exec_time_ns = res.exec_time_ns
    return out


kernel.last_exec_time_ns = None
